# revision 8
# baseline (speedup 1.0000x reference)
"""BitNet attention TRN2 kernel: 8-core SPMD (2 batch groups x 4 head groups).

Per core cid = 4*g + j (g = batch index, j = head-group index):
  - ternary-quantized QKV projections for heads [4j, 4j+4) of batch g
    (fp32r matmuls: TF32-like precision at full PE rate),
  - attention (scores fp32r, softmax with exact row max, fp32 denominators),
  - partial attn-mean accumulated in fp32, ReduceScattered over the 4-core
    batch group -> core owns rows [512j, 512(j+1)) of attn.mean,
  - attended values AllGathered over the batch group -> output projection for
    output columns [512j, 512(j+1)).
BitNet per-tensor scales: each core reduces |w| over a distinct 256-row slab
of each weight; one tiny 8-core AllReduce yields the full-tensor means.
"""

import os

import numpy as np

os.environ.setdefault("NEURON_RT_RESET_CORES", "1")

B, S, D, H = 2, 2048, 2048, 16
HD = D // H            # 128 head dim
HG = H // 4            # 4 heads per core
OS = HG * HD           # 512-wide output slice per core
P = 128
NCORES = 8
NDT = D // P           # 16 contraction tiles
C_SCALE = np.float32(1.0 / np.sqrt(HD))
THRESH = np.float32(2.0 / 3.0)

_CACHE = {}


def _build(use_mask: bool):
    import concourse.mybir as mybir
    import concourse.tile as tile
    from concourse import bacc
    from concourse.masks import make_identity

    F32 = mybir.dt.float32
    F32R = mybir.dt.float32r
    I32 = mybir.dt.int32
    AX = mybir.AxisListType
    ALU = mybir.AluOpType
    ACTF = mybir.ActivationFunctionType

    nc = bacc.Bacc("TRN2", target_bir_lowering=False, debug=False,
                   num_devices=NCORES)

    # ---- I/O ----
    xq_d = nc.dram_tensor("xq", [S, D], F32, kind="ExternalInput")
    xk_d = nc.dram_tensor("xk", [S, D], F32, kind="ExternalInput")
    xv_d = nc.dram_tensor("xv", [S, D], F32, kind="ExternalInput")
    wslab_d = nc.dram_tensor("w_slab", [4, 256, D], F32, kind="ExternalInput")
    w_in = {
        "q": nc.dram_tensor("wq_s", [OS, D], F32, kind="ExternalInput"),
        "k": nc.dram_tensor("wk_s", [OS, D], F32, kind="ExternalInput"),
        "v": nc.dram_tensor("wv_s", [OS, D], F32, kind="ExternalInput"),
        "o": nc.dram_tensor("wo_s", [OS, D], F32, kind="ExternalInput"),
    }
    bq_d = nc.dram_tensor("bq_s", [P, HG], F32, kind="ExternalInput")
    bk_d = nc.dram_tensor("bk_s", [P, HG], F32, kind="ExternalInput")
    bv_d = nc.dram_tensor("bv_s", [P, HG], F32, kind="ExternalInput")
    bo_d = nc.dram_tensor("bo_s", [1, OS], F32, kind="ExternalInput")
    if use_mask:
        mask_d = nc.dram_tensor("mask_g", [1, S], I32, kind="ExternalInput")
    out_d = nc.dram_tensor("out_slice", [S, OS], F32, kind="ExternalOutput")
    mean_d = nc.dram_tensor("mean_slice", [OS, S], F32, kind="ExternalOutput")

    groups8 = [[0, 1, 2, 3, 4, 5, 6, 7]]
    groups4 = [[0, 1, 2, 3], [4, 5, 6, 7]]
    WIDX = {"q": 0, "k": 1, "v": 2, "o": 3}

    with tile.TileContext(nc) as tc:
        with tc.tile_pool(name="dram", bufs=1, space="DRAM") as dram, \
             tc.tile_pool(name="const", bufs=1) as const:

            # internal DRAM staging
            cc_in = dram.tile([4], F32)
            cc_out = dram.tile([4], F32)
            qT_dram = dram.tile([HG, P, S], F32R)
            attT_part = [dram.tile([OS, 512], F32R, name=f"attT_part{i}")
                         for i in range(4)]
            attT_full = [dram.tile([S, 512], F32R, name=f"attT_full{i}")
                         for i in range(4)]
            mean_part = dram.tile([S, S], F32)
            mean_rs = dram.tile([OS, S], F32)

            # constants
            ident_f = const.tile([P, P], F32)
            make_identity(nc, ident_f[:])
            ident_r = const.tile([P, P], F32R)
            nc.vector.tensor_copy(out=ident_r[:], in_=ident_f[:])
            ones128 = const.tile([P, 1], F32)
            nc.vector.memset(ones128[:], 1.0)
            ones1f = const.tile([1, P], F32)
            nc.vector.memset(ones1f[:], 1.0)
            ones1r = const.tile([1, P], F32R)
            nc.vector.tensor_copy(out=ones1r[:], in_=ones1f[:])

            bias_sb = {}
            for nm, d in (("q", bq_d), ("k", bk_d), ("v", bv_d)):
                t = const.tile([P, HG], F32, name=f"bias_{nm}")
                nc.sync.dma_start(out=t[:], in_=d.ap()[:])
                bias_sb[nm] = t
            bo_row = const.tile([1, OS], F32)
            nc.sync.dma_start(out=bo_row[:], in_=bo_d.ap()[:])
            bo_row_r = const.tile([1, OS], F32R)
            nc.scalar.copy(out=bo_row_r[:], in_=bo_row[:])

            # ---------- Phase W: |w| slab sums -> AllReduce -> scales ----------
            acc4 = const.tile([P, 4], F32)
            with tc.tile_pool(name="slab", bufs=2) as slabp, \
                 tc.tile_pool(name="w0psum", bufs=1, space="PSUM") as w0p:
                for wi in range(4):
                    sl = slabp.tile([P, 2, D], F32, tag="slab")
                    nc.sync.dma_start(
                        out=sl[:],
                        in_=wslab_d.ap()[wi].rearrange("(ss p) d -> p ss d",
                                                       p=P))
                    dummy = slabp.tile([P, 2, D], F32, tag="dummy")
                    nc.scalar.activation(dummy[:], sl[:], ACTF.Abs,
                                         accum_out=acc4[:, wi:wi + 1])
                ps4 = w0p.tile([4, 1], F32, tag="ps4")
                nc.tensor.matmul(ps4[:], acc4[:], ones128[:], start=True,
                                 stop=True)
                sums_sb = const.tile([4, 1], F32)
                nc.scalar.copy(out=sums_sb[:], in_=ps4[:])
            nc.sync.dma_start(out=cc_in[:], in_=sums_sb[:])
            nc.gpsimd.collective_compute(
                "AllReduce", ALU.add, replica_groups=groups8,
                ins=[cc_in[:]], outs=[cc_out[:]])
            rsum = const.tile([1, 4], F32)
            nc.sync.dma_start(out=rsum[:], in_=cc_out[:])

            scale4 = const.tile([1, 4], F32)
            nc.vector.tensor_scalar(out=scale4[:], in0=rsum[:],
                                    scalar1=float(np.float32(1.0 / (D * D))),
                                    scalar2=1e-5, op0=ALU.mult, op1=ALU.max)
            nc.vector.tensor_scalar(out=scale4[:], in0=scale4[:],
                                    scalar1=1000.0, scalar2=None, op0=ALU.min)
            thr4 = const.tile([1, 4], F32)
            nc.vector.tensor_scalar(out=thr4[:], in0=scale4[:],
                                    scalar1=float(THRESH), scalar2=None,
                                    op0=ALU.mult)
            nthr4 = const.tile([1, 4], F32)
            nc.vector.tensor_scalar(out=nthr4[:], in0=thr4[:], scalar1=-1.0,
                                    scalar2=None, op0=ALU.mult)
            scale_c4 = const.tile([1, 4], F32)
            nc.vector.tensor_scalar(out=scale_c4[:], in0=scale4[:],
                                    scalar1=float(C_SCALE), scalar2=None,
                                    op0=ALU.mult)

            def bcast(src_ap, name):
                t = const.tile([P, 1], F32, name=name)
                nc.gpsimd.partition_broadcast(t[:], src_ap)
                return t

            thr_bc = [bcast(thr4[:, wi:wi + 1], f"thr{wi}")
                      for wi in range(4)]
            nthr_bc = [bcast(nthr4[:, wi:wi + 1], f"nthr{wi}")
                       for wi in range(4)]
            sc_bc = [bcast(scale4[:, wi:wi + 1], f"sc{wi}")
                     for wi in range(4)]
            scq_bc = bcast(scale_c4[:, 0:1], "scqc")

            if use_mask:
                mrow = const.tile([1, S], I32)
                nc.sync.dma_start(out=mrow[:], in_=mask_d.ap()[:])
                mb1 = const.tile([1, S], F32)
                nc.vector.tensor_scalar(out=mb1[:], in0=mrow[:], scalar1=-1.0,
                                        scalar2=1e9, op0=ALU.add, op1=ALU.mult)
                mbias = const.tile([P, S], F32)
                nc.gpsimd.partition_broadcast(mbias[:], mb1[:])

            # ---------- ternarize one weight, chunked by 128 o-rows ----------
            # tern = ((w >= -t) - 1) + (w > t)  in {-1, 0, 1}
            def ternarize(nm, wT_tile, psum_pool, scratch):
                wi = WIDX[nm]
                for os_i in range(HG):
                    wnat = scratch.tile([P, D], F32, tag="wnat")
                    nc.sync.dma_start(
                        out=wnat[:],
                        in_=w_in[nm].ap()[os_i * P:(os_i + 1) * P, :])
                    tmp = scratch.tile([P, D], F32, tag="terntmp")
                    nc.vector.tensor_scalar(out=tmp[:], in0=wnat[:],
                                            scalar1=nthr_bc[wi][:],
                                            scalar2=-1.0, op0=ALU.is_ge,
                                            op1=ALU.add)
                    gt = scratch.tile([P, D], F32, tag="terngt")
                    nc.vector.tensor_scalar(out=gt[:], in0=wnat[:],
                                            scalar1=thr_bc[wi][:],
                                            scalar2=None, op0=ALU.is_gt)
                    tern = scratch.tile([P, D], F32R, tag="tern")
                    nc.vector.tensor_tensor(out=tern[:], in0=tmp[:],
                                            in1=gt[:], op=ALU.add)
                    for dt_g in range(4):
                        pt = psum_pool.tile([P, 512], F32R, tag="wtp", bufs=2)
                        for di in range(4):
                            dt_i = dt_g * 4 + di
                            nc.tensor.transpose(
                                pt[:, di * P:(di + 1) * P],
                                tern[:, dt_i * P:(dt_i + 1) * P],
                                ident_r[:])
                        if dt_g % 2 == 0:
                            nc.scalar.copy(
                                out=wT_tile[:, dt_g * 4:dt_g * 4 + 4,
                                            os_i * P:(os_i + 1) * P],
                                in_=pt[:].bitcast(F32).rearrange(
                                    "p (di o) -> p di o", di=4))
                        else:
                            nc.vector.tensor_copy(
                                out=wT_tile[:, dt_g * 4:dt_g * 4 + 4,
                                            os_i * P:(os_i + 1) * P],
                                in_=pt[:].bitcast(F32).rearrange(
                                    "p (di o) -> p di o", di=4))

            with tc.tile_pool(name="kv", bufs=1) as kvp:
                kT_sb = kvp.tile([P, HG, S], F32R)        # [d', h, s]
                v_sb = kvp.tile([P, 16, OS], F32R)        # [s_p, st, o]

                # ---------- Phase X: projections ----------
                with tc.tile_pool(name="wt", bufs=1) as wtp, \
                     tc.tile_pool(name="xnat", bufs=1) as xnatp, \
                     tc.tile_pool(name="xt", bufs=1) as xtp, \
                     tc.tile_pool(name="qstage", bufs=3) as qstg, \
                     tc.tile_pool(name="wscratch", bufs=1) as wscr, \
                     tc.tile_pool(name="pxt", bufs=2, space="PSUM") as pxt, \
                     tc.tile_pool(name="pmm", bufs=2, space="PSUM") as pmm:

                    for nm, x_d in (("q", xq_d), ("k", xk_d), ("v", xv_d)):
                        wT = wtp.tile([P, NDT, OS], F32R, tag="wT")
                        ternarize(nm, wT, pxt, wscr)
                        for sb in range(4):
                            xnat = xnatp.tile([P, 4, D], F32, tag="xnat")
                            for ss in range(4):
                                r0 = sb * 512 + ss * P
                                nc.sync.dma_start(
                                    out=xnat[:, ss, :],
                                    in_=x_d.ap()[r0:r0 + P, :])
                            xT = xtp.tile([P, NDT, 512], F32R, tag="xT")
                            for dt_i in range(NDT):
                                pt = pxt.tile([P, 512], F32, tag="xtp")
                                for ss in range(4):
                                    nc.tensor.transpose(
                                        pt[:, ss * P:(ss + 1) * P],
                                        xnat[:, ss, dt_i * P:(dt_i + 1) * P],
                                        ident_f[:])
                                nc.scalar.copy(out=xT[:, dt_i, :], in_=pt[:])
                            if nm in ("q", "k"):
                                for ot in range(HG):
                                    pp = pmm.tile([P, 512], F32, tag="pp")
                                    for dt_i in range(NDT):
                                        nc.tensor.matmul(
                                            pp[:],
                                            wT[:, dt_i, ot * P:(ot + 1) * P],
                                            xT[:, dt_i, :],
                                            start=(dt_i == 0),
                                            stop=(dt_i == NDT - 1))
                                    if nm == "q":
                                        st = qstg.tile([P, 512], F32R,
                                                       tag="qs")
                                        nc.scalar.activation(
                                            st[:], pp[:], ACTF.Identity,
                                            bias=bias_sb["q"][:, ot:ot + 1],
                                            scale=scq_bc[:])
                                        nc.sync.dma_start(
                                            out=qT_dram[ot, :,
                                                        sb * 512:
                                                        (sb + 1) * 512],
                                            in_=st[:])
                                    else:
                                        nc.scalar.activation(
                                            kT_sb[:, ot,
                                                  sb * 512:(sb + 1) * 512],
                                            pp[:], ACTF.Identity,
                                            bias=bias_sb["k"][:, ot:ot + 1],
                                            scale=sc_bc[1][:])
                            else:
                                for st_i in range(4):
                                    pp = pmm.tile([P, OS], F32, tag="pp")
                                    for dt_i in range(NDT):
                                        nc.tensor.matmul(
                                            pp[:],
                                            xT[:, dt_i,
                                               st_i * P:(st_i + 1) * P],
                                            wT[:, dt_i, :],
                                            start=(dt_i == 0),
                                            stop=(dt_i == NDT - 1))
                                    nc.scalar.activation(
                                        v_sb[:, sb * 4 + st_i, :], pp[:],
                                        ACTF.Copy, scale=sc_bc[2][:])

                # ---------- Phase A: attention ----------
                with tc.tile_pool(name="accp", bufs=1) as accp, \
                     tc.tile_pool(name="ptld", bufs=1) as ptld, \
                     tc.tile_pool(name="probs", bufs=2) as probsp, \
                     tc.tile_pool(name="qsl", bufs=4) as qslp, \
                     tc.tile_pool(name="attts", bufs=2) as atttp, \
                     tc.tile_pool(name="smax", bufs=4) as smaxp, \
                     tc.tile_pool(name="scp", bufs=4, space="PSUM") as scp, \
                     tc.tile_pool(name="tp", bufs=2, space="PSUM") as tpp, \
                     tc.tile_pool(name="avp", bufs=2, space="PSUM") as avp:

                    for qb in range(4):
                        acc = accp.tile([P, 4, S], F32, tag="acc")
                        attT_sb = atttp.tile([P, HG, 512], F32R, tag="attT")
                        for h in range(HG):
                            probsT = ptld.tile([P, 16, 512], F32R, tag="pT")
                            for qt in range(4):
                                qsl = qslp.tile([P, P], F32R, tag="qsl")
                                q0 = (qb * 4 + qt) * P
                                nc.sync.dma_start(
                                    out=qsl[:], in_=qT_dram[h, :, q0:q0 + P])
                                psc = [scp.tile([P, 512], F32, tag="sc",
                                                name=f"sc{kb}")
                                       for kb in range(4)]
                                for kb in range(4):
                                    nc.tensor.matmul(
                                        psc[kb][:], qsl[:],
                                        kT_sb[:, h, kb * 512:(kb + 1) * 512],
                                        start=True, stop=True)
                                if use_mask:
                                    for kb in range(4):
                                        nc.vector.tensor_tensor(
                                            out=psc[kb][:], in0=psc[kb][:],
                                            in1=mbias[:,
                                                      kb * 512:(kb + 1) * 512],
                                            op=ALU.add)
                                nm4 = smaxp.tile([P, 4], F32, tag="nm4")
                                for kb in range(4):
                                    nc.vector.tensor_reduce(
                                        out=nm4[:, kb:kb + 1], in_=psc[kb][:],
                                        axis=AX.X, op=ALU.max)
                                nmax = smaxp.tile([P, 1], F32, tag="nmax")
                                nc.vector.tensor_reduce(
                                    out=nmax[:], in_=nm4[:], axis=AX.X,
                                    op=ALU.max, negate=True)
                                probs = probsp.tile([P, S], F32R, tag="probs")
                                den4 = smaxp.tile([P, 4], F32, tag="den4")
                                for kb in range(4):
                                    nc.scalar.activation(
                                        probs[:, kb * 512:(kb + 1) * 512],
                                        psc[kb][:], ACTF.Exp, bias=nmax[:],
                                        scale=1.0,
                                        accum_out=den4[:, kb:kb + 1])
                                den = smaxp.tile([P, 1], F32, tag="den")
                                nc.vector.tensor_reduce(
                                    out=den[:], in_=den4[:], axis=AX.X,
                                    op=ALU.add)
                                den16 = smaxp.tile([P, 1], F32, tag="den16")
                                nc.vector.tensor_scalar(
                                    out=den16[:], in0=den[:], scalar1=16.0,
                                    scalar2=None, op0=ALU.mult)
                                r16 = smaxp.tile([P, 1], F32, tag="r16")
                                nc.vector.reciprocal(out=r16[:], in_=den16[:])
                                nc.vector.tensor_scalar(
                                    out=probs[:], in0=probs[:].bitcast(F32),
                                    scalar1=r16[:], scalar2=None,
                                    op0=ALU.mult)
                                if h == 0:
                                    nc.vector.tensor_copy(
                                        out=acc[:, qt, :],
                                        in_=probs[:].bitcast(F32))
                                else:
                                    nc.vector.tensor_tensor(
                                        out=acc[:, qt, :], in0=acc[:, qt, :],
                                        in1=probs[:].bitcast(F32), op=ALU.add)
                                for tt in range(4):
                                    pt = tpp.tile([P, 512], F32R, tag="tp")
                                    for ki in range(4):
                                        kt = tt * 4 + ki
                                        nc.tensor.transpose(
                                            pt[:, ki * P:(ki + 1) * P],
                                            probs[:, kt * P:(kt + 1) * P],
                                            ident_r[:])
                                    src = pt[:].bitcast(F32).rearrange(
                                        "p (ki q) -> p ki q", ki=4)
                                    dst = probsT[:, tt * 4:tt * 4 + 4,
                                                 qt * P:(qt + 1) * P]
                                    if tt % 2 == 0:
                                        nc.scalar.copy(out=dst, in_=src)
                                    else:
                                        nc.vector.tensor_copy(out=dst,
                                                              in_=src)
                            pav = avp.tile([P, 512], F32, tag="av")
                            for kt in range(16):
                                nc.tensor.matmul(
                                    pav[:], v_sb[:, kt, h * P:(h + 1) * P],
                                    probsT[:, kt, :],
                                    start=(kt == 0), stop=(kt == 15))
                            nc.vector.tensor_scalar(
                                out=attT_sb[:, h, :], in0=pav[:],
                                scalar1=16.0,
                                scalar2=bias_sb["v"][:, h:h + 1],
                                op0=ALU.mult, op1=ALU.add)
                        nc.sync.dma_start(
                            out=attT_part[qb][:].rearrange(
                                "(h p) q -> p h q", p=P),
                            in_=attT_sb[:])
                        nc.gpsimd.collective_compute(
                            "AllGather", ALU.bypass, replica_groups=groups4,
                            ins=[attT_part[qb][:]], outs=[attT_full[qb][:]])
                        nc.sync.dma_start(
                            out=mean_part[qb * 512:(qb + 1) * 512, :]
                            .rearrange("(qt p) k -> p qt k", p=P),
                            in_=acc[:])

                    nc.gpsimd.collective_compute(
                        "ReduceScatter", ALU.add, replica_groups=groups4,
                        ins=[mean_part[:]], outs=[mean_rs[:]])
                    nc.sync.dma_start(out=mean_d.ap()[:], in_=mean_rs[:])

            # ---------- Phase O: wo ternarize + output projection ----------
            with tc.tile_pool(name="wo", bufs=1) as wop, \
                 tc.tile_pool(name="oscratch", bufs=1) as oscr, \
                 tc.tile_pool(name="attc", bufs=2) as attcp, \
                 tc.tile_pool(name="outs", bufs=2) as outsp, \
                 tc.tile_pool(name="pop", bufs=4, space="PSUM") as pop:
                woT = wop.tile([P, NDT, OS], F32R, tag="woT")
                ternarize("o", woT, pop, oscr)
                for qb in range(4):
                    for st_i in range(4):
                        attc = attcp.tile([P, NDT, P], F32R, tag="attc")
                        nc.sync.dma_start(
                            out=attc[:],
                            in_=attT_full[qb][:, st_i * P:(st_i + 1) * P]
                            .rearrange("(dt p) s -> p dt s", p=P))
                        po = pop.tile([P, OS], F32, tag="po")
                        for dt_i in range(NDT):
                            nc.tensor.matmul(
                                po[:], attc[:, dt_i, :], woT[:, dt_i, :],
                                start=(dt_i == 0), stop=False)
                        nc.tensor.matmul(po[:], ones1r[:], bo_row_r[:],
                                         start=False, stop=True)
                        osb = outsp.tile([P, OS], F32, tag="osb")
                        nc.scalar.activation(osb[:], po[:], ACTF.Copy,
                                             scale=sc_bc[3][:])
                        r0 = (qb * 4 + st_i) * P
                        nc.sync.dma_start(out=out_d.ap()[r0:r0 + P, :],
                                          in_=osb[:])

    nc.compile()
    return nc


def kernel(**inputs):
    query = np.ascontiguousarray(inputs["query"], dtype=np.float32)
    key = np.ascontiguousarray(inputs["key"], dtype=np.float32)
    value = np.ascontiguousarray(inputs["value"], dtype=np.float32)
    mask = np.asarray(inputs["mask"])
    ws = {n: np.ascontiguousarray(inputs[n], dtype=np.float32)
          for n in ("wq", "wk", "wv", "wo")}
    bs = {n: np.ascontiguousarray(inputs[n], dtype=np.float32)
          for n in ("bq", "bk", "bv", "bo")}

    use_mask = not bool(np.all(mask == 1))
    if use_mask not in _CACHE:
        _CACHE[use_mask] = _build(use_mask)
    nc = _CACHE[use_mask]

    in_maps = []
    for cid in range(NCORES):
        g, j = divmod(cid, 4)
        sl = slice(OS * j, OS * (j + 1))
        m = {
            "xq": query[g], "xk": key[g], "xv": value[g],
            "w_slab": np.stack([ws[n][256 * cid:256 * (cid + 1), :]
                                for n in ("wq", "wk", "wv", "wo")]),
            "wq_s": ws["wq"][sl], "wk_s": ws["wk"][sl],
            "wv_s": ws["wv"][sl], "wo_s": ws["wo"][sl],
            "bq_s": np.ascontiguousarray(bs["bq"][sl].reshape(HG, P).T),
            "bk_s": np.ascontiguousarray(bs["bk"][sl].reshape(HG, P).T),
            "bv_s": np.ascontiguousarray(bs["bv"][sl].reshape(HG, P).T),
            "bo_s": bs["bo"][sl].reshape(1, OS),
        }
        if use_mask:
            m["mask_g"] = np.ascontiguousarray(
                mask[g], dtype=np.int32).reshape(1, S)
        in_maps.append(m)

    from concourse.bass_utils import run_bass_kernel_spmd
    res = run_bass_kernel_spmd(nc, in_maps, core_ids=list(range(NCORES)))

    out = np.empty((B, S, D), np.float32)
    attn_mean = np.empty((B, S, S), np.float32)
    for cid in range(NCORES):
        g, j = divmod(cid, 4)
        out[g][:, OS * j:OS * (j + 1)] = res.results[cid]["out_slice"]
        attn_mean[g][OS * j:OS * (j + 1), :] = res.results[cid]["mean_slice"]
    return out, attn_mean


# revision 15
# speedup vs baseline: 141.7727x; 141.7727x over previous
"""BitNet attention TRN2 kernel: 8-core SPMD (2 batch groups x 4 head groups).

Per core cid = 4*g + j (g = batch index, j = head-group index):
  - ternary-quantized QKV projections for heads [4j, 4j+4) of batch g
    (fp32r matmuls: TF32-like precision at full PE rate),
  - attention (scores fp32r, softmax with exact row max, fp32 denominators),
  - partial attn-mean accumulated in fp32, ReduceScattered over the 4-core
    batch group -> core owns rows [512j, 512(j+1)) of attn.mean,
  - attended values AllGathered over the batch group -> output projection for
    output columns [512j, 512(j+1)).
BitNet per-tensor scales: each core reduces |w| over a distinct 256-row slab
of each weight; one tiny 8-core AllReduce yields the full-tensor means.
"""

import os

import numpy as np

os.environ.setdefault("NEURON_RT_RESET_CORES", "1")

B, S, D, H = 2, 2048, 2048, 16
HD = D // H            # 128 head dim
HG = H // 4            # 4 heads per core
OS = HG * HD           # 512-wide output slice per core
P = 128
NCORES = 8
NDT = D // P           # 16 contraction tiles
C_SCALE = np.float32(1.0 / np.sqrt(HD))
THRESH = np.float32(2.0 / 3.0)

_CACHE = {}


def _build(use_mask: bool):
    import concourse.mybir as mybir
    import concourse.tile as tile
    from concourse import bacc
    from concourse.masks import make_identity

    F32 = mybir.dt.float32
    F32R = mybir.dt.float32r
    BF16 = mybir.dt.bfloat16
    I32 = mybir.dt.int32
    AX = mybir.AxisListType
    ALU = mybir.AluOpType
    ACTF = mybir.ActivationFunctionType

    nc = bacc.Bacc("TRN2", target_bir_lowering=False, debug=False,
                   num_devices=NCORES)

    # ---- I/O ----
    xq_d = nc.dram_tensor("xq", [S, D], F32, kind="ExternalInput")
    xk_d = nc.dram_tensor("xk", [S, D], F32, kind="ExternalInput")
    xv_d = nc.dram_tensor("xv", [S, D], F32, kind="ExternalInput")
    wslab_d = nc.dram_tensor("w_slab", [4, 256, D], F32, kind="ExternalInput")
    w_in = {
        "q": nc.dram_tensor("wq_s", [OS, D], F32, kind="ExternalInput"),
        "k": nc.dram_tensor("wk_s", [OS, D], F32, kind="ExternalInput"),
        "v": nc.dram_tensor("wv_s", [OS, D], F32, kind="ExternalInput"),
        "o": nc.dram_tensor("wo_s", [OS, D], F32, kind="ExternalInput"),
    }
    bq_d = nc.dram_tensor("bq_s", [P, HG], F32, kind="ExternalInput")
    bk_d = nc.dram_tensor("bk_s", [P, HG], F32, kind="ExternalInput")
    bv_d = nc.dram_tensor("bv_s", [P, HG], F32, kind="ExternalInput")
    bo_d = nc.dram_tensor("bo_s", [1, OS], F32, kind="ExternalInput")
    if use_mask:
        mask_d = nc.dram_tensor("mask_g", [1, S], I32, kind="ExternalInput")
    out_d = nc.dram_tensor("out_slice", [S, OS], F32, kind="ExternalOutput")
    mean_d = nc.dram_tensor("mean_slice", [OS, S], F32, kind="ExternalOutput")

    groups8 = [[0, 1, 2, 3, 4, 5, 6, 7]]
    groups4 = [[0, 1, 2, 3], [4, 5, 6, 7]]
    WIDX = {"q": 0, "k": 1, "v": 2, "o": 3}

    with tile.TileContext(nc) as tc:
        with tc.tile_pool(name="dram", bufs=1, space="DRAM") as dram, \
             tc.tile_pool(name="const", bufs=1) as const:

            # internal DRAM staging
            cc_in = dram.tile([4], F32)
            cc_out = dram.tile([4], F32)
            qT_hi_dram = dram.tile([HG, P, S], BF16)
            qT_lo_dram = dram.tile([HG, P, S], BF16)
            attT_part = [dram.tile([OS, 512], F32R, name=f"attT_part{i}")
                         for i in range(4)]
            attT_full = [dram.tile([S, 512], F32R, name=f"attT_full{i}")
                         for i in range(4)]
            mean_part = dram.tile([S, S], F32)
            mean_rs = dram.tile([OS, S], F32)

            # constants
            ident_f = const.tile([P, P], F32)
            make_identity(nc, ident_f[:])
            ident_r = const.tile([P, P], F32R)
            nc.vector.tensor_copy(out=ident_r[:], in_=ident_f[:])
            ones128 = const.tile([P, 1], F32)
            nc.vector.memset(ones128[:], 1.0)
            ones1f = const.tile([1, P], F32)
            nc.vector.memset(ones1f[:], 1.0)
            ones1r = const.tile([1, P], F32R)
            nc.vector.tensor_copy(out=ones1r[:], in_=ones1f[:])

            bias_sb = {}
            for nm, d in (("q", bq_d), ("k", bk_d), ("v", bv_d)):
                t = const.tile([P, HG], F32, name=f"bias_{nm}")
                nc.sync.dma_start(out=t[:], in_=d.ap()[:])
                bias_sb[nm] = t
            bo_row = const.tile([1, OS], F32)
            nc.sync.dma_start(out=bo_row[:], in_=bo_d.ap()[:])
            bo_row_r = const.tile([1, OS], F32R)
            nc.scalar.copy(out=bo_row_r[:], in_=bo_row[:])

            # ---------- Phase W: |w| slab sums -> AllReduce -> scales ----------
            acc4 = const.tile([P, 4], F32)
            with tc.tile_pool(name="slab", bufs=2) as slabp, \
                 tc.tile_pool(name="w0psum", bufs=1, space="PSUM") as w0p:
                for wi in range(4):
                    sl = slabp.tile([P, 2, D], F32, tag="slab")
                    nc.sync.dma_start(
                        out=sl[:],
                        in_=wslab_d.ap()[wi].rearrange("(ss p) d -> p ss d",
                                                       p=P))
                    dummy = slabp.tile([P, 2, D], F32, tag="dummy")
                    nc.scalar.activation(dummy[:], sl[:], ACTF.Abs,
                                         accum_out=acc4[:, wi:wi + 1])
                ps4 = w0p.tile([4, 1], F32, tag="ps4")
                nc.tensor.matmul(ps4[:], acc4[:], ones128[:], start=True,
                                 stop=True)
                sums_sb = const.tile([4, 1], F32)
                nc.scalar.copy(out=sums_sb[:], in_=ps4[:])
            nc.sync.dma_start(out=cc_in[:], in_=sums_sb[:])
            nc.gpsimd.collective_compute(
                "AllReduce", ALU.add, replica_groups=groups8,
                ins=[cc_in[:]], outs=[cc_out[:]])
            rsum = const.tile([1, 4], F32)
            nc.sync.dma_start(out=rsum[:], in_=cc_out[:])

            scale4 = const.tile([1, 4], F32)
            nc.vector.tensor_scalar(out=scale4[:], in0=rsum[:],
                                    scalar1=float(np.float32(1.0 / (D * D))),
                                    scalar2=1e-5, op0=ALU.mult, op1=ALU.max)
            nc.vector.tensor_scalar(out=scale4[:], in0=scale4[:],
                                    scalar1=1000.0, scalar2=None, op0=ALU.min)
            thr4 = const.tile([1, 4], F32)
            nc.vector.tensor_scalar(out=thr4[:], in0=scale4[:],
                                    scalar1=float(THRESH), scalar2=None,
                                    op0=ALU.mult)
            nthr4 = const.tile([1, 4], F32)
            nc.vector.tensor_scalar(out=nthr4[:], in0=thr4[:], scalar1=-1.0,
                                    scalar2=None, op0=ALU.mult)
            scale_c4 = const.tile([1, 4], F32)
            nc.vector.tensor_scalar(out=scale_c4[:], in0=scale4[:],
                                    scalar1=float(C_SCALE), scalar2=None,
                                    op0=ALU.mult)

            def bcast(src_ap, name):
                t = const.tile([P, 1], F32, name=name)
                nc.gpsimd.partition_broadcast(t[:], src_ap)
                return t

            thr_bc = [bcast(thr4[:, wi:wi + 1], f"thr{wi}")
                      for wi in range(4)]
            nthr_bc = [bcast(nthr4[:, wi:wi + 1], f"nthr{wi}")
                       for wi in range(4)]
            sc_bc = [bcast(scale4[:, wi:wi + 1], f"sc{wi}")
                     for wi in range(4)]
            scq_bc = bcast(scale_c4[:, 0:1], "scqc")

            if use_mask:
                mrow = const.tile([1, S], I32)
                nc.sync.dma_start(out=mrow[:], in_=mask_d.ap()[:])
                mb1 = const.tile([1, S], F32)
                nc.vector.tensor_scalar(out=mb1[:], in0=mrow[:], scalar1=-1.0,
                                        scalar2=1e9, op0=ALU.add, op1=ALU.mult)
                mbias = const.tile([P, S], F32)
                nc.gpsimd.partition_broadcast(mbias[:], mb1[:])

            # ---------- ternarize one weight, chunked by 128 o-rows ----------
            # tern = ((w >= -t) - 1) + (w > t)  in {-1, 0, 1}
            def ternarize(nm, wT_tile, psum_pool, scratch, dtype):
                wi = WIDX[nm]
                HD2 = D // 2
                for os_i in range(HG):
                    for dh in range(2):
                        dsl = slice(dh * HD2, (dh + 1) * HD2)
                        wnat = scratch.tile([P, HD2], F32, tag="wnat")
                        nc.sync.dma_start(
                            out=wnat[:],
                            in_=w_in[nm].ap()[os_i * P:(os_i + 1) * P, dsl])
                        tmp = scratch.tile([P, HD2], F32, tag="terntmp")
                        nc.vector.tensor_scalar(out=tmp[:], in0=wnat[:],
                                                scalar1=nthr_bc[wi][:],
                                                scalar2=-1.0, op0=ALU.is_ge,
                                                op1=ALU.add)
                        gt = scratch.tile([P, HD2], F32, tag="terngt")
                        nc.vector.tensor_scalar(out=gt[:], in0=wnat[:],
                                                scalar1=thr_bc[wi][:],
                                                scalar2=None, op0=ALU.is_gt)
                        tern = scratch.tile([P, HD2], dtype, tag="tern")
                        nc.vector.tensor_tensor(out=tern[:], in0=tmp[:],
                                                in1=gt[:], op=ALU.add)
                        dt0 = dh * 8
                        if dtype == BF16:
                            # 2-byte: xbar DMA transpose, no PE involvement
                            nc.sync.dma_start_transpose(
                                out=wT_tile[:, dt0:dt0 + 8,
                                            os_i * P:(os_i + 1) * P],
                                in_=tern[:])
                            continue
                        for dt_g in range(2):
                            pt = psum_pool.tile([P, 512], F32R, tag="wtp",
                                                bufs=2)
                            for di in range(4):
                                dt_i = dt_g * 4 + di
                                nc.tensor.transpose(
                                    pt[:, di * P:(di + 1) * P],
                                    tern[:, dt_i * P:(dt_i + 1) * P],
                                    ident_r[:])
                            dtg0 = dt0 + dt_g * 4
                            if dt_g % 2 == 0:
                                nc.scalar.copy(
                                    out=wT_tile[:, dtg0:dtg0 + 4,
                                                os_i * P:(os_i + 1) * P],
                                    in_=pt[:].bitcast(F32).rearrange(
                                        "p (di o) -> p di o", di=4))
                            else:
                                nc.vector.tensor_copy(
                                    out=wT_tile[:, dtg0:dtg0 + 4,
                                                os_i * P:(os_i + 1) * P],
                                    in_=pt[:].bitcast(F32).rearrange(
                                        "p (di o) -> p di o", di=4))

            with tc.tile_pool(name="kv", bufs=1) as kvp:
                kT_hi = kvp.tile([P, HG, S], BF16)        # [d', h, s]
                kT_lo = kvp.tile([P, HG, S], BF16)
                v_sb = kvp.tile([P, 16, OS], F32R)        # [s_p, st, o]

                # ---------- Phase X: projections ----------
                with tc.tile_pool(name="wt", bufs=1) as wtp, \
                     tc.tile_pool(name="xnat", bufs=1) as xnatp, \
                     tc.tile_pool(name="xt", bufs=1) as xtp, \
                     tc.tile_pool(name="qstage", bufs=2) as qstg, \
                     tc.tile_pool(name="wscratch", bufs=1) as wscr, \
                     tc.tile_pool(name="pxt", bufs=2, space="PSUM") as pxt, \
                     tc.tile_pool(name="pmm", bufs=2, space="PSUM") as pmm:

                    for nm, x_d in (("q", xq_d), ("k", xk_d), ("v", xv_d)):
                        hilo = nm in ("q", "k")
                        wdt = BF16 if hilo else F32R
                        wT = wtp.tile([P, NDT, OS], wdt, tag="wT",
                                      name=f"wT_{nm}")
                        ternarize(nm, wT, pxt, wscr, wdt)
                        for sb in range(4):
                            xnat = xnatp.tile([P, 4, D], F32, tag="xnat")
                            for ss in range(4):
                                r0 = sb * 512 + ss * P
                                nc.sync.dma_start(
                                    out=xnat[:, ss, :],
                                    in_=x_d.ap()[r0:r0 + P, :])
                            if hilo:
                                xTh = xtp.tile([P, NDT, 512], BF16,
                                               tag="xTh")
                                xTl = xtp.tile([P, NDT, 512], BF16,
                                               tag="xTl")
                            else:
                                xT = xtp.tile([P, NDT, 512], F32R, tag="xTh")
                            for dt_i in range(NDT):
                                pt = pxt.tile([P, 512], F32, tag="xtp")
                                for ss in range(4):
                                    nc.tensor.transpose(
                                        pt[:, ss * P:(ss + 1) * P],
                                        xnat[:, ss, dt_i * P:(dt_i + 1) * P],
                                        ident_f[:])
                                if hilo:
                                    nc.scalar.copy(out=xTh[:, dt_i, :],
                                                   in_=pt[:])
                                    nc.vector.tensor_tensor(
                                        out=xTl[:, dt_i, :], in0=pt[:],
                                        in1=xTh[:, dt_i, :],
                                        op=ALU.subtract)
                                else:
                                    nc.scalar.copy(out=xT[:, dt_i, :],
                                                   in_=pt[:])
                            if hilo:
                                for ot in range(HG):
                                    pp = pmm.tile([P, 512], F32, tag="pp")
                                    for dt_i in range(NDT):
                                        nc.tensor.matmul(
                                            pp[:],
                                            wT[:, dt_i, ot * P:(ot + 1) * P],
                                            xTh[:, dt_i, :],
                                            start=(dt_i == 0), stop=False)
                                    for dt_i in range(NDT):
                                        nc.tensor.matmul(
                                            pp[:],
                                            wT[:, dt_i, ot * P:(ot + 1) * P],
                                            xTl[:, dt_i, :],
                                            start=False,
                                            stop=(dt_i == NDT - 1))
                                    ev = qstg.tile([P, 512], F32, tag="ev")
                                    if nm == "q":
                                        nc.scalar.activation(
                                            ev[:], pp[:], ACTF.Identity,
                                            bias=bias_sb["q"][:, ot:ot + 1],
                                            scale=scq_bc[:])
                                    else:
                                        nc.scalar.activation(
                                            ev[:], pp[:], ACTF.Identity,
                                            bias=bias_sb["k"][:, ot:ot + 1],
                                            scale=sc_bc[1][:])
                                    if nm == "q":
                                        qh = qstg.tile([P, 512], BF16,
                                                       tag="qh")
                                        ql = qstg.tile([P, 512], BF16,
                                                       tag="ql")
                                        nc.scalar.copy(out=qh[:], in_=ev[:])
                                        nc.vector.tensor_tensor(
                                            out=ql[:], in0=ev[:], in1=qh[:],
                                            op=ALU.subtract)
                                        sl5 = slice(sb * 512, (sb + 1) * 512)
                                        nc.sync.dma_start(
                                            out=qT_hi_dram[ot, :, sl5],
                                            in_=qh[:])
                                        nc.sync.dma_start(
                                            out=qT_lo_dram[ot, :, sl5],
                                            in_=ql[:])
                                    else:
                                        sl5 = slice(sb * 512, (sb + 1) * 512)
                                        nc.scalar.copy(
                                            out=kT_hi[:, ot, sl5], in_=ev[:])
                                        nc.vector.tensor_tensor(
                                            out=kT_lo[:, ot, sl5],
                                            in0=ev[:], in1=kT_hi[:, ot, sl5],
                                            op=ALU.subtract)
                            else:
                                for st_i in range(4):
                                    pp = pmm.tile([P, OS], F32, tag="pp")
                                    for dt_i in range(NDT):
                                        nc.tensor.matmul(
                                            pp[:],
                                            xT[:, dt_i,
                                               st_i * P:(st_i + 1) * P],
                                            wT[:, dt_i, :],
                                            start=(dt_i == 0),
                                            stop=(dt_i == NDT - 1))
                                    nc.scalar.activation(
                                        v_sb[:, sb * 4 + st_i, :], pp[:],
                                        ACTF.Copy, scale=sc_bc[2][:])

                # ---------- Phase A: attention ----------
                with tc.tile_pool(name="accp", bufs=1) as accp, \
                     tc.tile_pool(name="ptld", bufs=1) as ptld, \
                     tc.tile_pool(name="probs", bufs=2) as probsp, \
                     tc.tile_pool(name="qsl", bufs=4) as qslp, \
                     tc.tile_pool(name="attts", bufs=2) as atttp, \
                     tc.tile_pool(name="smax", bufs=4) as smaxp, \
                     tc.tile_pool(name="scp", bufs=4, space="PSUM") as scp, \
                     tc.tile_pool(name="tp", bufs=2, space="PSUM") as tpp, \
                     tc.tile_pool(name="avp", bufs=2, space="PSUM") as avp:

                    for qb in range(4):
                        acc = accp.tile([P, 4, S], F32, tag="acc")
                        attT_sb = atttp.tile([P, HG, 512], F32R, tag="attT")
                        for h in range(HG):
                            probsT = ptld.tile([P, 16, 512], F32R, tag="pT")
                            for qt in range(4):
                                qslh = qslp.tile([P, P], BF16, tag="qslh")
                                qsll = qslp.tile([P, P], BF16, tag="qsll")
                                q0 = (qb * 4 + qt) * P
                                nc.sync.dma_start(
                                    out=qslh[:],
                                    in_=qT_hi_dram[h, :, q0:q0 + P])
                                nc.sync.dma_start(
                                    out=qsll[:],
                                    in_=qT_lo_dram[h, :, q0:q0 + P])
                                psc = [scp.tile([P, 512], F32, tag="sc",
                                                name=f"sc{kb}")
                                       for kb in range(4)]
                                for kb in range(4):
                                    kbs = slice(kb * 512, (kb + 1) * 512)
                                    nc.tensor.matmul(
                                        psc[kb][:], qslh[:],
                                        kT_hi[:, h, kbs],
                                        start=True, stop=False)
                                    nc.tensor.matmul(
                                        psc[kb][:], qslh[:],
                                        kT_lo[:, h, kbs],
                                        start=False, stop=False)
                                    nc.tensor.matmul(
                                        psc[kb][:], qsll[:],
                                        kT_hi[:, h, kbs],
                                        start=False, stop=True)
                                if use_mask:
                                    for kb in range(4):
                                        nc.vector.tensor_tensor(
                                            out=psc[kb][:], in0=psc[kb][:],
                                            in1=mbias[:,
                                                      kb * 512:(kb + 1) * 512],
                                            op=ALU.add)
                                nm4 = smaxp.tile([P, 4], F32, tag="nm4")
                                for kb in range(4):
                                    nc.vector.tensor_reduce(
                                        out=nm4[:, kb:kb + 1], in_=psc[kb][:],
                                        axis=AX.X, op=ALU.max)
                                nmax = smaxp.tile([P, 1], F32, tag="nmax")
                                nc.vector.tensor_reduce(
                                    out=nmax[:], in_=nm4[:], axis=AX.X,
                                    op=ALU.max, negate=True)
                                probs = probsp.tile([P, S], F32R, tag="probs")
                                den4 = smaxp.tile([P, 4], F32, tag="den4")
                                for kb in range(4):
                                    nc.scalar.activation(
                                        probs[:, kb * 512:(kb + 1) * 512],
                                        psc[kb][:], ACTF.Exp, bias=nmax[:],
                                        scale=1.0,
                                        accum_out=den4[:, kb:kb + 1])
                                den = smaxp.tile([P, 1], F32, tag="den")
                                nc.vector.tensor_reduce(
                                    out=den[:], in_=den4[:], axis=AX.X,
                                    op=ALU.add)
                                den16 = smaxp.tile([P, 1], F32, tag="den16")
                                nc.vector.tensor_scalar(
                                    out=den16[:], in0=den[:], scalar1=16.0,
                                    scalar2=None, op0=ALU.mult)
                                r16 = smaxp.tile([P, 1], F32, tag="r16")
                                nc.vector.reciprocal(out=r16[:], in_=den16[:])
                                nc.vector.tensor_scalar(
                                    out=probs[:], in0=probs[:].bitcast(F32),
                                    scalar1=r16[:], scalar2=None,
                                    op0=ALU.mult)
                                if h == 0:
                                    nc.vector.tensor_copy(
                                        out=acc[:, qt, :],
                                        in_=probs[:].bitcast(F32))
                                else:
                                    nc.vector.tensor_tensor(
                                        out=acc[:, qt, :], in0=acc[:, qt, :],
                                        in1=probs[:].bitcast(F32), op=ALU.add)
                                for tt in range(4):
                                    pt = tpp.tile([P, 512], F32R, tag="tp")
                                    for ki in range(4):
                                        kt = tt * 4 + ki
                                        nc.tensor.transpose(
                                            pt[:, ki * P:(ki + 1) * P],
                                            probs[:, kt * P:(kt + 1) * P],
                                            ident_r[:])
                                    src = pt[:].bitcast(F32).rearrange(
                                        "p (ki q) -> p ki q", ki=4)
                                    dst = probsT[:, tt * 4:tt * 4 + 4,
                                                 qt * P:(qt + 1) * P]
                                    if tt % 2 == 0:
                                        nc.scalar.copy(out=dst, in_=src)
                                    else:
                                        nc.vector.tensor_copy(out=dst,
                                                              in_=src)
                            pav = avp.tile([P, 512], F32, tag="av")
                            for kt in range(16):
                                nc.tensor.matmul(
                                    pav[:], v_sb[:, kt, h * P:(h + 1) * P],
                                    probsT[:, kt, :],
                                    start=(kt == 0), stop=(kt == 15))
                            nc.vector.tensor_scalar(
                                out=attT_sb[:, h, :], in0=pav[:],
                                scalar1=16.0,
                                scalar2=bias_sb["v"][:, h:h + 1],
                                op0=ALU.mult, op1=ALU.add)
                        nc.sync.dma_start(
                            out=attT_part[qb][:].rearrange(
                                "(h p) q -> p h q", p=P),
                            in_=attT_sb[:])
                        nc.gpsimd.collective_compute(
                            "AllGather", ALU.bypass, replica_groups=groups4,
                            ins=[attT_part[qb][:]], outs=[attT_full[qb][:]])
                        nc.sync.dma_start(
                            out=mean_part[qb * 512:(qb + 1) * 512, :]
                            .rearrange("(qt p) k -> p qt k", p=P),
                            in_=acc[:])

                    nc.gpsimd.collective_compute(
                        "ReduceScatter", ALU.add, replica_groups=groups4,
                        ins=[mean_part[:]], outs=[mean_rs[:]])
                    nc.sync.dma_start(out=mean_d.ap()[:], in_=mean_rs[:])

            # ---------- Phase O: wo ternarize + output projection ----------
            with tc.tile_pool(name="wo", bufs=1) as wop, \
                 tc.tile_pool(name="oscratch", bufs=1) as oscr, \
                 tc.tile_pool(name="attc", bufs=2) as attcp, \
                 tc.tile_pool(name="outs", bufs=2) as outsp, \
                 tc.tile_pool(name="pop", bufs=4, space="PSUM") as pop:
                woT = wop.tile([P, NDT, OS], F32R, tag="woT")
                ternarize("o", woT, pop, oscr, F32R)
                for qb in range(4):
                    for st_i in range(4):
                        attc = attcp.tile([P, NDT, P], F32R, tag="attc")
                        nc.sync.dma_start(
                            out=attc[:],
                            in_=attT_full[qb][:, st_i * P:(st_i + 1) * P]
                            .rearrange("(dt p) s -> p dt s", p=P))
                        po = pop.tile([P, OS], F32, tag="po")
                        for dt_i in range(NDT):
                            nc.tensor.matmul(
                                po[:], attc[:, dt_i, :], woT[:, dt_i, :],
                                start=(dt_i == 0), stop=False)
                        nc.tensor.matmul(po[:], ones1r[:], bo_row_r[:],
                                         start=False, stop=True)
                        osb = outsp.tile([P, OS], F32, tag="osb")
                        nc.scalar.activation(osb[:], po[:], ACTF.Copy,
                                             scale=sc_bc[3][:])
                        r0 = (qb * 4 + st_i) * P
                        nc.sync.dma_start(out=out_d.ap()[r0:r0 + P, :],
                                          in_=osb[:])

    nc.compile()
    return nc


def kernel(**inputs):
    query = np.ascontiguousarray(inputs["query"], dtype=np.float32)
    key = np.ascontiguousarray(inputs["key"], dtype=np.float32)
    value = np.ascontiguousarray(inputs["value"], dtype=np.float32)
    mask = np.asarray(inputs["mask"])
    ws = {n: np.ascontiguousarray(inputs[n], dtype=np.float32)
          for n in ("wq", "wk", "wv", "wo")}
    bs = {n: np.ascontiguousarray(inputs[n], dtype=np.float32)
          for n in ("bq", "bk", "bv", "bo")}

    use_mask = not bool(np.all(mask == 1))
    if use_mask not in _CACHE:
        _CACHE[use_mask] = _build(use_mask)
    nc = _CACHE[use_mask]

    in_maps = []
    for cid in range(NCORES):
        g, j = divmod(cid, 4)
        sl = slice(OS * j, OS * (j + 1))
        m = {
            "xq": query[g], "xk": key[g], "xv": value[g],
            "w_slab": np.stack([ws[n][256 * cid:256 * (cid + 1), :]
                                for n in ("wq", "wk", "wv", "wo")]),
            "wq_s": ws["wq"][sl], "wk_s": ws["wk"][sl],
            "wv_s": ws["wv"][sl], "wo_s": ws["wo"][sl],
            "bq_s": np.ascontiguousarray(bs["bq"][sl].reshape(HG, P).T),
            "bk_s": np.ascontiguousarray(bs["bk"][sl].reshape(HG, P).T),
            "bv_s": np.ascontiguousarray(bs["bv"][sl].reshape(HG, P).T),
            "bo_s": bs["bo"][sl].reshape(1, OS),
        }
        if use_mask:
            m["mask_g"] = np.ascontiguousarray(
                mask[g], dtype=np.int32).reshape(1, S)
        in_maps.append(m)

    global _last_in_maps
    _last_in_maps = in_maps

    from concourse.bass_utils import run_bass_kernel_spmd
    res = run_bass_kernel_spmd(nc, in_maps, core_ids=list(range(NCORES)))

    out = np.empty((B, S, D), np.float32)
    attn_mean = np.empty((B, S, S), np.float32)
    for cid in range(NCORES):
        g, j = divmod(cid, 4)
        out[g][:, OS * j:OS * (j + 1)] = res.results[cid]["out_slice"]
        attn_mean[g][OS * j:OS * (j + 1), :] = res.results[cid]["mean_slice"]
    return out, attn_mean


# revision 27
# speedup vs baseline: 5672.1359x; 40.0087x over previous
"""BitNet attention TRN2 kernel: 8-core SPMD (2 batch groups x 4 head groups).

Per core cid = 4*g + j (g = batch index, j = head-group index):
  - ternary-quantized QKV projections for heads [4j, 4j+4) of batch g
    (fp32r matmuls: TF32-like precision at full PE rate),
  - attention (scores fp32r, softmax with exact row max, fp32 denominators),
  - partial attn-mean accumulated in fp32, ReduceScattered over the 4-core
    batch group -> core owns rows [512j, 512(j+1)) of attn.mean,
  - attended values AllGathered over the batch group -> output projection for
    output columns [512j, 512(j+1)).
BitNet per-tensor scales: each core reduces |w| over a distinct 256-row slab
of each weight; one tiny 8-core AllReduce yields the full-tensor means.
"""

import os

import numpy as np

os.environ.setdefault("NEURON_RT_RESET_CORES", "1")

B, S, D, H = 2, 2048, 2048, 16
HD = D // H            # 128 head dim
HG = H // 4            # 4 heads per core
OS = HG * HD           # 512-wide output slice per core
P = 128
NCORES = 8
NDT = D // P           # 16 contraction tiles
C_SCALE = np.float32(1.0 / np.sqrt(HD))
THRESH = np.float32(2.0 / 3.0)

_CACHE = {}


class _PhaseStop(Exception):
    def __init__(self, nc):
        self.nc = nc


def _build(use_mask: bool, single: bool = False, phases: int = 3):
    import concourse.mybir as mybir
    import concourse.tile as tile
    from concourse import bacc
    from concourse.masks import make_identity

    F32 = mybir.dt.float32
    F32R = mybir.dt.float32r
    BF16 = mybir.dt.bfloat16
    I32 = mybir.dt.int32
    AX = mybir.AxisListType
    ALU = mybir.AluOpType
    ACTF = mybir.ActivationFunctionType

    nc = bacc.Bacc("TRN2", target_bir_lowering=False, debug=False,
                   num_devices=1 if single else NCORES)

    def cc(kind, op, groups, ins, outs):
        if not single:
            nc.gpsimd.collective_compute(kind, op, replica_groups=groups,
                                         ins=ins, outs=outs)
            return
        # timing-only single-core substitute: local DMA of this core's part
        src_ap, dst_ap = ins[0], outs[0]
        if kind == "AllGather":
            nc.gpsimd.dma_start(out=dst_ap[0:src_ap.shape[0]], in_=src_ap)
        elif kind == "ReduceScatter":
            nc.gpsimd.dma_start(out=dst_ap, in_=src_ap[0:dst_ap.shape[0]])
        else:
            nc.gpsimd.dma_start(out=dst_ap, in_=src_ap)

    # ---- I/O ----
    xq_d = nc.dram_tensor("xq", [S, D], F32, kind="ExternalInput")
    xk_d = nc.dram_tensor("xk", [S, D], F32, kind="ExternalInput")
    xv_d = nc.dram_tensor("xv", [S, D], F32, kind="ExternalInput")
    wslab_d = nc.dram_tensor("w_slab", [4, 256, D], F32, kind="ExternalInput")
    w_in = {
        "q": nc.dram_tensor("wq_s", [OS, D], F32, kind="ExternalInput"),
        "k": nc.dram_tensor("wk_s", [OS, D], F32, kind="ExternalInput"),
        "v": nc.dram_tensor("wv_s", [OS, D], F32, kind="ExternalInput"),
        "o": nc.dram_tensor("wo_s", [OS, D], F32, kind="ExternalInput"),
    }
    bq_d = nc.dram_tensor("bq_s", [P, HG], F32, kind="ExternalInput")
    bk_d = nc.dram_tensor("bk_s", [P, HG], F32, kind="ExternalInput")
    bv_d = nc.dram_tensor("bv_s", [P, HG], F32, kind="ExternalInput")
    bo_d = nc.dram_tensor("bo_s", [1, OS], F32, kind="ExternalInput")
    if use_mask:
        mask_d = nc.dram_tensor("mask_g", [1, S], I32, kind="ExternalInput")
    out_d = nc.dram_tensor("out_slice", [S, OS], F32, kind="ExternalOutput")
    mean_d = nc.dram_tensor("mean_slice", [OS, S], F32, kind="ExternalOutput")

    groups8 = [[0, 1, 2, 3, 4, 5, 6, 7]]
    groups4 = [[0, 1, 2, 3], [4, 5, 6, 7]]
    WIDX = {"q": 0, "k": 1, "v": 2, "o": 3}

    with tile.TileContext(nc) as tc:
        with tc.tile_pool(name="dram", bufs=1, space="DRAM") as dram, \
             tc.tile_pool(name="const", bufs=1) as const:

            # internal DRAM staging
            cc_in = dram.tile([4], F32)
            cc_out = dram.tile([4], F32)
            qT_hi_dram = dram.tile([HG, P, S], BF16)
            qT_lo_dram = dram.tile([HG, P, S], BF16)
            attT_part = [dram.tile([OS, 512], F32R, name=f"attT_part{i}")
                         for i in range(4)]
            attT_full = [dram.tile([S, 512], F32R, name=f"attT_full{i}")
                         for i in range(4)]
            mean_part = dram.tile([S, S], F32)
            mean_rs = dram.tile([OS, S], F32)

            # constants
            ident_f = const.tile([P, P], F32)
            make_identity(nc, ident_f[:])
            ident_r = const.tile([P, P], F32R)
            nc.vector.tensor_copy(out=ident_r[:], in_=ident_f[:])
            ones128 = const.tile([P, 1], F32)
            nc.vector.memset(ones128[:], 1.0)
            ones1f = const.tile([1, P], F32)
            nc.vector.memset(ones1f[:], 1.0)
            ones1r = const.tile([1, P], F32R)
            nc.vector.tensor_copy(out=ones1r[:], in_=ones1f[:])

            bias_sb = {}
            for nm, d in (("q", bq_d), ("k", bk_d), ("v", bv_d)):
                t = const.tile([P, HG], F32, name=f"bias_{nm}")
                nc.sync.dma_start(out=t[:], in_=d.ap()[:])
                bias_sb[nm] = t
            bo_row = const.tile([1, OS], F32)
            nc.sync.dma_start(out=bo_row[:], in_=bo_d.ap()[:])
            bo_row_r = const.tile([1, OS], F32R)
            nc.scalar.copy(out=bo_row_r[:], in_=bo_row[:])

            # ---------- Phase W: |w| slab sums -> AllReduce -> scales ----------
            acc4 = const.tile([P, 4], F32)
            with tc.tile_pool(name="slab", bufs=2) as slabp, \
                 tc.tile_pool(name="w0psum", bufs=1, space="PSUM") as w0p:
                for wi in range(4):
                    sl = slabp.tile([P, 2, D], F32, tag="slab")
                    nc.sync.dma_start(
                        out=sl[:],
                        in_=wslab_d.ap()[wi].rearrange("(ss p) d -> p ss d",
                                                       p=P))
                    dummy = slabp.tile([P, 2, D], F32, tag="dummy")
                    nc.scalar.activation(dummy[:], sl[:], ACTF.Abs,
                                         accum_out=acc4[:, wi:wi + 1])
                ps4 = w0p.tile([4, 1], F32, tag="ps4")
                nc.tensor.matmul(ps4[:], acc4[:], ones128[:], start=True,
                                 stop=True)
                sums_sb = const.tile([4, 1], F32)
                nc.scalar.copy(out=sums_sb[:], in_=ps4[:])
            nc.sync.dma_start(out=cc_in[:], in_=sums_sb[:])
            cc("AllReduce", ALU.add, groups8, [cc_in[:]], [cc_out[:]])
            rsum = const.tile([1, 4], F32)
            nc.sync.dma_start(out=rsum[:], in_=cc_out[:])

            scale4 = const.tile([1, 4], F32)
            nc.vector.tensor_scalar(out=scale4[:], in0=rsum[:],
                                    scalar1=float(np.float32(1.0 / (D * D))),
                                    scalar2=1e-5, op0=ALU.mult, op1=ALU.max)
            nc.vector.tensor_scalar(out=scale4[:], in0=scale4[:],
                                    scalar1=1000.0, scalar2=None, op0=ALU.min)
            thr4 = const.tile([1, 4], F32)
            nc.vector.tensor_scalar(out=thr4[:], in0=scale4[:],
                                    scalar1=float(THRESH), scalar2=None,
                                    op0=ALU.mult)
            nthr4 = const.tile([1, 4], F32)
            nc.vector.tensor_scalar(out=nthr4[:], in0=thr4[:], scalar1=-1.0,
                                    scalar2=None, op0=ALU.mult)
            scale_c4 = const.tile([1, 4], F32)
            nc.vector.tensor_scalar(out=scale_c4[:], in0=scale4[:],
                                    scalar1=float(C_SCALE), scalar2=None,
                                    op0=ALU.mult)

            def bcast(src_ap, name):
                t = const.tile([P, 1], F32, name=name)
                nc.gpsimd.partition_broadcast(t[:], src_ap)
                return t

            thr_bc = [bcast(thr4[:, wi:wi + 1], f"thr{wi}")
                      for wi in range(4)]
            nthr_bc = [bcast(nthr4[:, wi:wi + 1], f"nthr{wi}")
                       for wi in range(4)]
            sc_bc = [bcast(scale4[:, wi:wi + 1], f"sc{wi}")
                     for wi in range(4)]
            scq_bc = bcast(scale_c4[:, 0:1], "scqc")

            if use_mask:
                mrow = const.tile([1, S], I32)
                nc.sync.dma_start(out=mrow[:], in_=mask_d.ap()[:])
                mb1 = const.tile([1, S], F32)
                nc.vector.tensor_scalar(out=mb1[:], in0=mrow[:], scalar1=-1.0,
                                        scalar2=1e9, op0=ALU.add, op1=ALU.mult)
                mbias = const.tile([P, S], F32)
                nc.gpsimd.partition_broadcast(mbias[:], mb1[:])

            # ---------- ternarize one weight, chunked by 128 o-rows ----------
            # tern = ((w >= -t) - 1) + (w > t)  in {-1, 0, 1}
            def ternarize(nm, wT_tile, psum_pool, scratch, dtype):
                wi = WIDX[nm]
                HD2 = D // 2
                for os_i in range(HG):
                    for dh in range(2):
                        dsl = slice(dh * HD2, (dh + 1) * HD2)
                        wnat = scratch.tile([P, HD2], F32, tag="wnat")
                        nc.sync.dma_start(
                            out=wnat[:],
                            in_=w_in[nm].ap()[os_i * P:(os_i + 1) * P, dsl])
                        tmp = scratch.tile([P, HD2], F32, tag="terntmp")
                        nc.vector.tensor_scalar(out=tmp[:], in0=wnat[:],
                                                scalar1=nthr_bc[wi][:],
                                                scalar2=-1.0, op0=ALU.is_ge,
                                                op1=ALU.add)
                        gt = scratch.tile([P, HD2], F32, tag="terngt")
                        nc.vector.tensor_scalar(out=gt[:], in0=wnat[:],
                                                scalar1=thr_bc[wi][:],
                                                scalar2=None, op0=ALU.is_gt)
                        tern = scratch.tile([P, HD2], dtype, tag="tern")
                        nc.vector.tensor_tensor(out=tern[:], in0=tmp[:],
                                                in1=gt[:], op=ALU.add)
                        dt0 = dh * 8
                        if dtype == BF16:
                            # 2-byte: xbar DMA transpose, no PE involvement
                            nc.sync.dma_start_transpose(
                                out=wT_tile[:, dt0:dt0 + 4,
                                            os_i * P:(os_i + 1) * P],
                                in_=tern[:])
                            continue
                        for dt_g in range(1):
                            pt = psum_pool.tile([P, 512], F32R, tag="wtp",
                                                bufs=2)
                            for di in range(4):
                                dt_i = dt_g * 4 + di
                                nc.tensor.transpose(
                                    pt[:, di * P:(di + 1) * P],
                                    tern[:, dt_i * P:(dt_i + 1) * P],
                                    ident_r[:])
                            dtg0 = dt0 + dt_g * 4
                            if dt_g % 2 == 0:
                                nc.scalar.copy(
                                    out=wT_tile[:, dtg0:dtg0 + 4,
                                                os_i * P:(os_i + 1) * P],
                                    in_=pt[:].bitcast(F32).rearrange(
                                        "p (di o) -> p di o", di=4))
                            else:
                                nc.vector.tensor_copy(
                                    out=wT_tile[:, dtg0:dtg0 + 4,
                                                os_i * P:(os_i + 1) * P],
                                    in_=pt[:].bitcast(F32).rearrange(
                                        "p (di o) -> p di o", di=4))

            with tc.tile_pool(name="kv", bufs=1) as kvp:
                kT_hi = kvp.tile([P, HG, S], BF16)        # [d', h, s]
                kT_lo = kvp.tile([P, HG, S], BF16)
                v_sb = kvp.tile([P, 16, OS], F32R)        # [s_p, st, o]

                # ---------- Phase X: projections ----------
                with tc.tile_pool(name="wt", bufs=1) as wtp, \
                     tc.tile_pool(name="xnat", bufs=1) as xnatp, \
                     tc.tile_pool(name="xt", bufs=1) as xtp, \
                     tc.tile_pool(name="qstage", bufs=4) as qstg, \
                     tc.tile_pool(name="wscratch", bufs=1) as wscr, \
                     tc.tile_pool(name="pxt", bufs=4, space="PSUM") as pxt, \
                     tc.tile_pool(name="pmm", bufs=4, space="PSUM") as pmm:

                    for nm, x_d in (("q", xq_d), ("k", xk_d), ("v", xv_d)):
                        hilo = nm in ("q", "k")
                        wdt = BF16 if hilo else F32R
                        wT = wtp.tile([P, NDT, OS], wdt, tag="wT",
                                      name=f"wT_{nm}")
                        ternarize(nm, wT, pxt, wscr, wdt)
                        for sb in range(4):
                            if hilo:
                                xTh = xtp.tile([P, NDT, 512], BF16,
                                               tag="xTh")
                                xTl = xtp.tile([P, NDT, 512], BF16,
                                               tag="xTl")
                                for ss in range(4):
                                    r0 = sb * 512 + ss * P
                                    xc = xnatp.tile([P, D], F32, tag="xc")
                                    nc.sync.dma_start(
                                        out=xc[:],
                                        in_=x_d.ap()[r0:r0 + P, :])
                                    xch = xnatp.tile([P, D], BF16, tag="xch")
                                    nc.scalar.copy(out=xch[:], in_=xc[:])
                                    xcl = xnatp.tile([P, D], BF16, tag="xcl")
                                    nc.vector.tensor_tensor(
                                        out=xcl[:], in0=xc[:], in1=xch[:],
                                        op=ALU.subtract)
                                    ssl = slice(ss * P, (ss + 1) * P)
                                    nc.sync.dma_start_transpose(
                                        out=xTh[:, :, ssl], in_=xch[:])
                                    nc.sync.dma_start_transpose(
                                        out=xTl[:, :, ssl], in_=xcl[:])
                            else:
                                xT = xtp.tile([P, NDT, 512], F32R, tag="xTh")
                                for ss in range(4):
                                    r0 = sb * 512 + ss * P
                                    xc = xnatp.tile([P, D], F32, tag="xc")
                                    nc.sync.dma_start(
                                        out=xc[:],
                                        in_=x_d.ap()[r0:r0 + P, :])
                                    for dtg in range(4):
                                        pt = pxt.tile([P, 512], F32,
                                                      tag="xtp")
                                        for di in range(4):
                                            dt_i = dtg * 4 + di
                                            nc.tensor.transpose(
                                                pt[:, di * P:(di + 1) * P],
                                                xc[:,
                                                   dt_i * P:(dt_i + 1) * P],
                                                ident_f[:])
                                        nc.scalar.copy(
                                            out=xT[:, dtg * 4:dtg * 4 + 4,
                                                   ss * P:(ss + 1) * P],
                                            in_=pt[:].rearrange(
                                                "p (di s) -> p di s", di=4))
                            if hilo:
                                for ot in range(HG):
                                    pp = pmm.tile([P, 512], F32, tag="pp")
                                    for dt_i in range(NDT):
                                        nc.tensor.matmul(
                                            pp[:],
                                            wT[:, dt_i, ot * P:(ot + 1) * P],
                                            xTh[:, dt_i, :],
                                            start=(dt_i == 0), stop=False)
                                    for dt_i in range(NDT):
                                        nc.tensor.matmul(
                                            pp[:],
                                            wT[:, dt_i, ot * P:(ot + 1) * P],
                                            xTl[:, dt_i, :],
                                            start=False,
                                            stop=(dt_i == NDT - 1))
                                    ev = qstg.tile([P, 512], F32, tag="ev")
                                    if nm == "q":
                                        nc.scalar.activation(
                                            ev[:], pp[:], ACTF.Identity,
                                            bias=bias_sb["q"][:, ot:ot + 1],
                                            scale=scq_bc[:])
                                    else:
                                        nc.scalar.activation(
                                            ev[:], pp[:], ACTF.Identity,
                                            bias=bias_sb["k"][:, ot:ot + 1],
                                            scale=sc_bc[1][:])
                                    if nm == "q":
                                        qh = qstg.tile([P, 512], BF16,
                                                       tag="qh")
                                        ql = qstg.tile([P, 512], BF16,
                                                       tag="ql")
                                        nc.scalar.copy(out=qh[:], in_=ev[:])
                                        nc.vector.tensor_tensor(
                                            out=ql[:], in0=ev[:], in1=qh[:],
                                            op=ALU.subtract)
                                        sl5 = slice(sb * 512, (sb + 1) * 512)
                                        nc.sync.dma_start(
                                            out=qT_hi_dram[ot, :, sl5],
                                            in_=qh[:])
                                        nc.sync.dma_start(
                                            out=qT_lo_dram[ot, :, sl5],
                                            in_=ql[:])
                                    else:
                                        sl5 = slice(sb * 512, (sb + 1) * 512)
                                        nc.scalar.copy(
                                            out=kT_hi[:, ot, sl5], in_=ev[:])
                                        nc.vector.tensor_tensor(
                                            out=kT_lo[:, ot, sl5],
                                            in0=ev[:], in1=kT_hi[:, ot, sl5],
                                            op=ALU.subtract)
                            else:
                                for st_i in range(4):
                                    pp = pmm.tile([P, OS], F32, tag="pp")
                                    for dt_i in range(NDT):
                                        nc.tensor.matmul(
                                            pp[:],
                                            xT[:, dt_i,
                                               st_i * P:(st_i + 1) * P],
                                            wT[:, dt_i, :],
                                            start=(dt_i == 0),
                                            stop=(dt_i == NDT - 1))
                                    nc.scalar.activation(
                                        v_sb[:, sb * 4 + st_i, :], pp[:],
                                        ACTF.Copy, scale=sc_bc[2][:])

                # ---------- Phase A: attention ----------
                if phases < 2:
                    raise _PhaseStop(nc)
                with tc.tile_pool(name="accp", bufs=1) as accp, \
                     tc.tile_pool(name="ptld", bufs=1) as ptld, \
                     tc.tile_pool(name="probs", bufs=2) as probsp, \
                     tc.tile_pool(name="qsl", bufs=2) as qslp, \
                     tc.tile_pool(name="attts", bufs=2) as atttp, \
                     tc.tile_pool(name="smax", bufs=4) as smaxp, \
                     tc.tile_pool(name="scp", bufs=6, space="PSUM") as scp, \
                     tc.tile_pool(name="tp", bufs=1, space="PSUM") as tpp, \
                     tc.tile_pool(name="avp", bufs=1, space="PSUM") as avp:

                    for qb in range(4):
                        acc = accp.tile([P, 4, S], F32, tag="acc")
                        attT_sb = atttp.tile([P, HG, 512], F32R, tag="attT")
                        qsl5 = slice(qb * 512, (qb + 1) * 512)
                        qbh = qslp.tile([P, HG, 512], BF16, tag="qbh")
                        qbl = qslp.tile([P, HG, 512], BF16, tag="qbl")
                        nc.sync.dma_start(
                            out=qbh[:], in_=qT_hi_dram[:, :, qsl5]
                            .rearrange("h p q -> p h q"))
                        nc.sync.dma_start(
                            out=qbl[:], in_=qT_lo_dram[:, :, qsl5]
                            .rearrange("h p q -> p h q"))
                        for h in range(HG):
                            probsT = ptld.tile([P, 16, 512], F32R, tag="pT")
                            for qt in range(4):
                                qts = slice(qt * P, (qt + 1) * P)
                                psc = [scp.tile([P, 512], F32, tag="sc",
                                                name=f"sc{kb}")
                                       for kb in range(4)]
                                for vi, (lh, kt_sb) in enumerate(
                                        ((qbh, kT_hi), (qbh, kT_lo),
                                         (qbl, kT_hi))):
                                    for kb in range(4):
                                        kbs = slice(kb * 512, (kb + 1) * 512)
                                        nc.tensor.matmul(
                                            psc[kb][:], lh[:, h, qts],
                                            kt_sb[:, h, kbs],
                                            start=(vi == 0), stop=(vi == 2))
                                nm4 = smaxp.tile([P, 4], F32, tag="nm4")
                                for kb in range(4):
                                    if use_mask:
                                        nc.vector.tensor_tensor(
                                            out=psc[kb][:], in0=psc[kb][:],
                                            in1=mbias[:,
                                                      kb * 512:(kb + 1) * 512],
                                            op=ALU.add)
                                    nc.vector.tensor_reduce(
                                        out=nm4[:, kb:kb + 1],
                                        in_=psc[kb][:],
                                        axis=AX.X, op=ALU.max)
                                nmax = smaxp.tile([P, 1], F32, tag="nmax")
                                nc.vector.tensor_reduce(
                                    out=nmax[:], in_=nm4[:], axis=AX.X,
                                    op=ALU.max, negate=True)
                                probs = probsp.tile([P, S], F32R, tag="probs")
                                den4 = smaxp.tile([P, 4], F32, tag="den4")
                                for kb in range(4):
                                    nc.scalar.activation(
                                        probs[:, kb * 512:(kb + 1) * 512],
                                        psc[kb][:], ACTF.Exp, bias=nmax[:],
                                        scale=1.0,
                                        accum_out=den4[:, kb:kb + 1])
                                den16 = smaxp.tile([P, 1], F32, tag="den16")
                                nc.vector.tensor_reduce(
                                    out=den16[:], in_=den4[:], axis=AX.X,
                                    op=ALU.add)
                                nc.vector.tensor_scalar(
                                    out=den16[:], in0=den16[:], scalar1=16.0,
                                    scalar2=None, op0=ALU.mult)
                                r16 = smaxp.tile([P, 1], F32, tag="r16")
                                nc.vector.reciprocal(out=r16[:], in_=den16[:])
                                nc.vector.tensor_scalar(
                                    out=probs[:], in0=probs[:].bitcast(F32),
                                    scalar1=r16[:], scalar2=None,
                                    op0=ALU.mult)
                                if h == 0:
                                    nc.vector.tensor_copy(
                                        out=acc[:, qt, :],
                                        in_=probs[:].bitcast(F32))
                                else:
                                    nc.vector.tensor_tensor(
                                        out=acc[:, qt, :], in0=acc[:, qt, :],
                                        in1=probs[:].bitcast(F32), op=ALU.add)
                                for tt in range(4):
                                    pt = tpp.tile([P, 512], F32R, tag="tp")
                                    for ki in range(4):
                                        kt = tt * 4 + ki
                                        nc.tensor.transpose(
                                            pt[:, ki * P:(ki + 1) * P],
                                            probs[:, kt * P:(kt + 1) * P],
                                            ident_r[:])
                                    src = pt[:].bitcast(F32).rearrange(
                                        "p (ki q) -> p ki q", ki=4)
                                    dst = probsT[:, tt * 4:tt * 4 + 4,
                                                 qt * P:(qt + 1) * P]
                                    if tt % 2 == 0:
                                        nc.scalar.copy(out=dst, in_=src)
                                    else:
                                        nc.vector.tensor_copy(out=dst,
                                                              in_=src)
                            pav = avp.tile([P, 512], F32, tag="av")
                            for kt in range(16):
                                nc.tensor.matmul(
                                    pav[:], v_sb[:, kt, h * P:(h + 1) * P],
                                    probsT[:, kt, :],
                                    start=(kt == 0), stop=(kt == 15))
                            nc.vector.tensor_scalar(
                                out=attT_sb[:, h, :], in0=pav[:],
                                scalar1=16.0,
                                scalar2=bias_sb["v"][:, h:h + 1],
                                op0=ALU.mult, op1=ALU.add)
                        nc.sync.dma_start(
                            out=attT_part[qb][:].rearrange(
                                "(h p) q -> p h q", p=P),
                            in_=attT_sb[:])
                        cc("AllGather", ALU.bypass, groups4,
                           [attT_part[qb][:]], [attT_full[qb][:]])
                        nc.sync.dma_start(
                            out=mean_part[qb * 512:(qb + 1) * 512, :]
                            .rearrange("(qt p) k -> p qt k", p=P),
                            in_=acc[:])

                    cc("ReduceScatter", ALU.add, groups4,
                       [mean_part[:]], [mean_rs[:]])
                    nc.sync.dma_start(out=mean_d.ap()[:], in_=mean_rs[:])

            # ---------- Phase O: wo ternarize + output projection ----------
            if phases < 3:
                raise _PhaseStop(nc)
            with tc.tile_pool(name="wo", bufs=1) as wop, \
                 tc.tile_pool(name="oscratch", bufs=1) as oscr, \
                 tc.tile_pool(name="attc", bufs=3) as attcp, \
                 tc.tile_pool(name="outs", bufs=4) as outsp, \
                 tc.tile_pool(name="pop", bufs=4, space="PSUM") as pop:
                woT = wop.tile([P, NDT, OS], F32R, tag="woT")
                ternarize("o", woT, pop, oscr, F32R)
                for qb in range(4):
                    for st_i in range(4):
                        attc = attcp.tile([P, NDT, P], F32R, tag="attc")
                        nc.sync.dma_start(
                            out=attc[:],
                            in_=attT_full[qb][:, st_i * P:(st_i + 1) * P]
                            .rearrange("(dt p) s -> p dt s", p=P))
                        po = pop.tile([P, OS], F32, tag="po")
                        for dt_i in range(NDT):
                            nc.tensor.matmul(
                                po[:], attc[:, dt_i, :], woT[:, dt_i, :],
                                start=(dt_i == 0), stop=False)
                        nc.tensor.matmul(po[:], ones1r[:], bo_row_r[:],
                                         start=False, stop=True)
                        osb = outsp.tile([P, OS], F32, tag="osb")
                        nc.scalar.activation(osb[:], po[:], ACTF.Copy,
                                             scale=sc_bc[3][:])
                        r0 = (qb * 4 + st_i) * P
                        nc.sync.dma_start(out=out_d.ap()[r0:r0 + P, :],
                                          in_=osb[:])

    nc.compile()
    return nc


def kernel(**inputs):
    query = np.ascontiguousarray(inputs["query"], dtype=np.float32)
    key = np.ascontiguousarray(inputs["key"], dtype=np.float32)
    value = np.ascontiguousarray(inputs["value"], dtype=np.float32)
    mask = np.asarray(inputs["mask"])
    ws = {n: np.ascontiguousarray(inputs[n], dtype=np.float32)
          for n in ("wq", "wk", "wv", "wo")}
    bs = {n: np.ascontiguousarray(inputs[n], dtype=np.float32)
          for n in ("bq", "bk", "bv", "bo")}

    use_mask = not bool(np.all(mask == 1))
    if use_mask not in _CACHE:
        _CACHE[use_mask] = _build(use_mask)
    nc = _CACHE[use_mask]

    in_maps = []
    for cid in range(NCORES):
        g, j = divmod(cid, 4)
        sl = slice(OS * j, OS * (j + 1))
        m = {
            "xq": query[g], "xk": key[g], "xv": value[g],
            "w_slab": np.stack([ws[n][256 * cid:256 * (cid + 1), :]
                                for n in ("wq", "wk", "wv", "wo")]),
            "wq_s": ws["wq"][sl], "wk_s": ws["wk"][sl],
            "wv_s": ws["wv"][sl], "wo_s": ws["wo"][sl],
            "bq_s": np.ascontiguousarray(bs["bq"][sl].reshape(HG, P).T),
            "bk_s": np.ascontiguousarray(bs["bk"][sl].reshape(HG, P).T),
            "bv_s": np.ascontiguousarray(bs["bv"][sl].reshape(HG, P).T),
            "bo_s": bs["bo"][sl].reshape(1, OS),
        }
        if use_mask:
            m["mask_g"] = np.ascontiguousarray(
                mask[g], dtype=np.int32).reshape(1, S)
        in_maps.append(m)

    global _last_in_maps
    _last_in_maps = in_maps

    from concourse.bass_utils import run_bass_kernel_spmd
    res = run_bass_kernel_spmd(nc, in_maps, core_ids=list(range(NCORES)))

    out = np.empty((B, S, D), np.float32)
    attn_mean = np.empty((B, S, S), np.float32)
    for cid in range(NCORES):
        g, j = divmod(cid, 4)
        out[g][:, OS * j:OS * (j + 1)] = res.results[cid]["out_slice"]
        attn_mean[g][OS * j:OS * (j + 1), :] = res.results[cid]["mean_slice"]
    return out, attn_mean
